# revision 11
# baseline (speedup 1.0000x reference)
"""Multi-head attention (B=4, S=2048, D=768, 12 heads) on 8 TRN2 NeuronCores.

Sharding: data parallel over batch (4) x tensor parallel over heads (2 groups
of 6 heads) = 8 cores. Each core computes its (batch, head-group) slice.

v2 pipeline (fp8 flash attention):
  Q^T/K^T projections in [feat, seq] bf16 layout; V projected per seq-tile
  and evacuated to fp8 as (V8, Vr=V-V8) DoubleRow planes with a 0.125 ones
  column per head for the softmax denominator. Transposed scores S^T[k,q]
  per head pair with row-tiled concurrent K=64 bf16 matmuls; exp(0.125 s)*8
  written straight to fp8: the leading q-columns on the scalar engine
  (ACT exp), the tail on the vector engine via a Schraudolph int8 bit-trick
  (x*8/ln2 + bias, round-to-nearest, bitcast e4m3). P@V runs as fp8
  DoubleRow matmuls: lhsT = (V8, Vr) planes, rhs = pt broadcast to both
  planes (stride 0), accumulating O^T + denominator rows in PSUM. fp16
  reciprocal + K=1 fp16 broadcast matmul, normalization multiply on DVE
  (with the 1/8 pt-scale folded into the O evacuation).
Host side only reshapes/casts for sharding and un-transposes on gather.
"""

import numpy as np
import ml_dtypes

B, S, D = 4, 2048, 768
NH, HD = 12, 64
HPC = 6                 # heads per core
FPC = HPC * HD          # 384 features per core
VW = HPC * 65           # 390: V width with per-head denom column (PSUM)
VSLOT = 80              # fp8 V slot stride per head (16-aligned)
N_CORES = 8
BF16 = ml_dtypes.bfloat16

# exp constants: pt = 8*exp(0.125*s) in fp8 e4m3
PT_LN_SCALE = float(np.log(8.0))
SCHRAUD_MUL = 0.125 * 8.0 / float(np.log(2.0))   # 1.4427*0.125*8... = x*8/ln2 applied to 0.125*s
SCHRAUD_SIGMA = 0.6
SCHRAUD_BIAS = 8.0 * (3 + 7) - SCHRAUD_SIGMA     # scale 2^3, e4m3 bias 7
DVE_Q = 256             # of each 1024-col pt tile, last DVE_Q columns exp on
                        # DVE (head 1 only) -> dve share = DVE_Q/1024

_PROGRAM = None


def _build_program():
    import concourse.bass as bass  # noqa: F401
    import concourse.mybir as mybir
    from concourse import bacc
    from concourse.tile import TileContext
    from contextlib import ExitStack

    F = mybir.dt.float32
    F16 = mybir.dt.float16
    BF = mybir.dt.bfloat16
    F8 = mybir.dt.float8e4
    I8 = mybir.dt.int8
    EXP = mybir.ActivationFunctionType.Exp
    DR = mybir.MatmulPerfMode.DoubleRow

    nc = bacc.Bacc("TRN2", target_bir_lowering=False, debug=False, num_devices=N_CORES)

    xt = nc.dram_tensor("xt", [D, S], BF, kind="ExternalInput")
    wqt = nc.dram_tensor("wqt", [D, FPC], BF, kind="ExternalInput")
    wkt = nc.dram_tensor("wkt", [D, FPC], BF, kind="ExternalInput")
    wvt = nc.dram_tensor("wvt", [D, VW], BF, kind="ExternalInput")
    bqk = nc.dram_tensor("bqk", [FPC, 2], F, kind="ExternalInput")
    bv = nc.dram_tensor("bv", [1, VW], BF, kind="ExternalInput")
    out = nc.dram_tensor("out", [FPC, S], F, kind="ExternalOutput")

    KT = D // 128        # 6 contraction tiles for projections
    MT = FPC // 128      # 3 feature tiles (= head pairs)
    QC = S // 512        # 4 seq chunks of 512
    JT = S // 128        # 16 key tiles

    with TileContext(nc) as tc, ExitStack() as ctx:
        const = ctx.enter_context(tc.tile_pool(name="const", bufs=1))
        qkv = ctx.enter_context(tc.tile_pool(name="qkv", bufs=1))
        osb = ctx.enter_context(tc.tile_pool(name="osb", bufs=1))
        ppool = ctx.enter_context(tc.tile_pool(name="pt", bufs=4))
        small = ctx.enter_context(tc.tile_pool(name="small", bufs=4))
        ps_pr = ctx.enter_context(tc.tile_pool(name="pspr", bufs=2, space="PSUM"))
        ps_s = ctx.enter_context(tc.tile_pool(name="pss", bufs=2, space="PSUM"))
        ps_o = ctx.enter_context(tc.tile_pool(name="pso", bufs=1, space="PSUM"))

        # ---- stage inputs in SBUF with one consolidated DMA per tensor
        xt_all = const.tile([128, KT * S], BF, tag="xta", name="xta")
        xt_s = [xt_all[:, i * S:(i + 1) * S] for i in range(KT)]
        wq_all = const.tile([128, KT * FPC], BF, tag="wqa", name="wqa")
        wqt_s = [wq_all[:, i * FPC:(i + 1) * FPC] for i in range(KT)]
        wk_all = const.tile([128, KT * FPC], BF, tag="wka", name="wka")
        wkt_s = [wk_all[:, i * FPC:(i + 1) * FPC] for i in range(KT)]
        wv_all = const.tile([128, KT * VW], BF, tag="wva", name="wva")
        wvt_s = [wv_all[:, i * VW:(i + 1) * VW] for i in range(KT)]
        bqk_all = const.tile([128, MT * 2], F, tag="bqk", name="bqka")
        bq_s = [bqk_all[:, 2 * t_i:2 * t_i + 1] for t_i in range(MT)]
        bk_s = [bqk_all[:, 2 * t_i + 1:2 * t_i + 2] for t_i in range(MT)]
        bv_s = const.tile([1, VW], BF, tag="bv")

        nc.sync.dma_start(
            wq_all[:].rearrange("p (b c) -> p b c", b=KT),
            wqt[:].rearrange("(b p) c -> p b c", p=128))
        nc.sync.dma_start(
            wk_all[:].rearrange("p (b c) -> p b c", b=KT),
            wkt[:].rearrange("(b p) c -> p b c", p=128))
        nc.sync.dma_start(
            bqk_all[:].rearrange("p (t c) -> p t c", t=MT),
            bqk[:].rearrange("(t p) c -> p t c", p=128))
        ones_s = const.tile([1, 128], BF, tag="ones")
        nc.vector.memset(ones_s[:], 1.0)
        ones64 = const.tile([1, 64], F16, tag="ones64")
        nc.vector.memset(ones64[:], 1.0)
        ln8 = const.tile([128, 1], F, tag="ln8")
        nc.vector.memset(ln8[:], PT_LN_SCALE)

        # dummy exp so the ACT table set loads during the DMA prologue
        dummy = small.tile([1, 1], F, tag="dummy", name="dummy")
        nc.scalar.activation(dummy[:], bq_s[0][0:1, :], EXP)

        # fp8 V tiles: [128 seq, 2 planes (V8, Vr) x 6*VSLOT], ones col at 64
        VPW = HPC * VSLOT  # 480 bytes per plane
        v_s = [qkv.tile([128, 2 * VPW], F8, tag=f"v{m}", name=f"v{m}")
               for m in range(JT)]

        def v_plane(m, plane):
            # [128, 6, 65] view of one (V8|Vr) plane for evacuation
            return v_s[m][:, plane * VPW:(plane + 1) * VPW].rearrange(
                "p (h c) -> p h c", h=HPC)[:, :, 0:65]

        def v_pair(m, lh):
            # [128, 2, 65] DoubleRow lhsT for head-slot lh
            return v_s[m][:].rearrange("p (i w) -> p i w", i=2)[
                :, :, lh * VSLOT:lh * VSLOT + 65]
        qt_s = [qkv.tile([128, S], BF, tag=f"q{t_i}", name=f"qt{t_i}") for t_i in range(MT)]
        kt_s = [qkv.tile([128, S], BF, tag=f"k{t_i}", name=f"kt{t_i}") for t_i in range(MT)]
        o_s = [osb.tile([128, S], F, tag=f"o{t_i}", name=f"ot{t_i}") for t_i in range(MT)]

        # ---- projection work, broken into single-matmul thunks
        def v_group_thunks(m):
            # V projection in PSUM [128, 390], evac to fp8 (V8, Vr) planes
            cell = {}

            def mk(kk):
                def thunk():
                    if "ps" not in cell:
                        cell["ps"] = ps_pr.tile([128, 512], F, tag="pr",
                                                name=f"psv{m}")
                    ps65 = cell["ps"][:, 0:VW].rearrange(
                        "p (h c) -> p h c", h=HPC)
                    if kk < KT:
                        nc.tensor.matmul(
                            cell["ps"][:, 0:VW],
                            lhsT=xt_s[kk][:, m * 128:(m + 1) * 128],
                            rhs=wvt_s[kk][:], start=(kk == 0), stop=False)
                    elif kk == KT:
                        nc.tensor.matmul(cell["ps"][:, 0:VW], lhsT=ones_s[:],
                                         rhs=bv_s[:], start=False, stop=True)
                        nc.vector.tensor_copy(v_plane(m, 0), ps65)
                    else:
                        # Vr plane = PSUM - V8
                        nc.vector.tensor_sub(v_plane(m, 1), ps65,
                                             v_plane(m, 0))
                return thunk
            return [mk(kk) for kk in range(KT + 2)]

        def qk_group_thunks(w_s, b_s, dst, p, qc, use_act=False):
            cell = {}

            def mk(kk):
                def thunk():
                    if "ps" not in cell:
                        cell["ps"] = ps_pr.tile([128, 512], F, tag="pr",
                                                name=f"psp{p}_{qc}")
                    nc.tensor.matmul(
                        cell["ps"][:],
                        lhsT=w_s[kk][:, p * 128:(p + 1) * 128],
                        rhs=xt_s[kk][:, qc * 512:(qc + 1) * 512],
                        start=(kk == 0), stop=(kk == KT - 1))
                    if kk == KT - 1:
                        if use_act:
                            nc.scalar.add(
                                dst[p][:, qc * 512:(qc + 1) * 512],
                                cell["ps"][:], b_s[p][:])
                        else:
                            nc.vector.tensor_scalar_add(
                                dst[p][:, qc * 512:(qc + 1) * 512],
                                cell["ps"][:], b_s[p][:])
                return thunk
            return [mk(kk) for kk in range(KT)]

        # xt chunk DMAs via SWDGE in parallel with weight DMAs on HWDGE
        for qc in range(QC):
            nc.gpsimd.dma_start(
                xt_all[:].rearrange("p (b c) -> p b c", b=KT)[
                    :, :, qc * 512:(qc + 1) * 512],
                xt[:].rearrange("(b p) c -> p b c", p=128)[
                    :, :, qc * 512:(qc + 1) * 512])
        nc.sync.dma_start(
            wv_all[:].rearrange("p (b c) -> p b c", b=KT),
            wvt[:].rearrange("(b p) c -> p b c", p=128))
        nc.sync.dma_start(bv_s[:], bv[:])

        # prologue compute: pair-0 chunk-0 Q/K projections
        for w_s, b_s, dst in ((wqt_s, bq_s, qt_s), (wkt_s, bk_s, kt_s)):
            for th in qk_group_thunks(w_s, b_s, dst, 0, qc=0):
                th()

        from collections import deque
        items = []   # (deadline, order, thunk)

        def add(deadline, thunks):
            for th in thunks:
                items.append((deadline, len(items), th))

        for m in range(JT):
            add(m, v_group_thunks(m))                       # PV(0,0,m) at step m
        for qc in range(1, QC):
            add(max(0, 4 * qc - 2),
                qk_group_thunks(wkt_s, bk_s, kt_s, 0, qc))
            add(max(0, 16 * qc - 2),
                qk_group_thunks(wqt_s, bq_s, qt_s, 0, qc))
        for p in range(1, MT):
            base = 64 * p
            for qc in range(QC):
                add(base + 4 * qc - 2,
                    qk_group_thunks(wkt_s, bk_s, kt_s, p, qc))
                add(base + 16 * qc - 2,
                    qk_group_thunks(wqt_s, bq_s, qt_s, p, qc))
        items.sort(key=lambda x: (x[0], x[1]))
        proj_q = deque(items)

        # ---- attention pipeline over flattened (pair, q-chunk, k-tile) steps
        steps = [(p, qc, j) for p in range(MT) for qc in range(QC)
                 for j in range(JT)]

        def emit_scores(p, qc, j):
            sp = ps_s.tile([128, 1024], F, tag="s", name=f"s{p}_{qc}_{j}")
            for h in range(2):
                nc.tensor.matmul(
                    sp[:, h * 512:(h + 1) * 512],
                    lhsT=kt_s[p][h * 64:(h + 1) * 64, j * 128:(j + 1) * 128],
                    rhs=qt_s[p][h * 64:(h + 1) * 64, qc * 512:(qc + 1) * 512],
                    start=True, stop=True, tile_position=(h * 64, 0))
            return sp

        def make_norm(p, qc, osb, recs, s, final=False):
            # broadcast + normalize on GPSIMD (Pool) -- the idle engine --
            # freeing PE (no bc matmul) and DVE (no copy/mul)
            def norm():
                for h in range(2):
                    bc_sb = small.tile([64, 512], F, tag="bc", name=f"bcs{s}_{h}")
                    nc.gpsimd.partition_broadcast(bc_sb[:], recs[h][:],
                                                  channels=64)
                    nc.gpsimd.tensor_mul(
                        o_s[p][h * 64:(h + 1) * 64, qc * 512:(qc + 1) * 512],
                        osb[h][:], bc_sb[:])
                nc.sync.dma_start(
                    out[p * 128:(p + 1) * 128, qc * 512:(qc + 1) * 512],
                    o_s[p][:, qc * 512:(qc + 1) * 512])
            return norm

        sp_next = emit_scores(*steps[0])
        Os = None
        pending_norm = None
        for s, (p, qc, j) in enumerate(steps):
            sp = sp_next
            if s + 1 < len(steps):
                sp_next = emit_scores(*steps[s + 1])
            if pending_norm is not None:
                pending_norm()
                pending_norm = None
            extra = 2
            while proj_q and (proj_q[0][0] <= s or extra > 0):
                if proj_q[0][0] > s:
                    extra -= 1
                proj_q.popleft()[2]()
            if j == 0:
                O0 = ps_o.tile([65, 512], F, tag="o0", name=f"o0_{p}_{qc}")
                O1 = ps_o.tile([65, 512], F, tag="o1", name=f"o1_{p}_{qc}")
                Os = (O0, O1)
            # exp -> fp8 pt: ACT takes the first 1024-DVE_Q columns, DVE
            # (Schraudolph int8 bit-trick) the rest
            pt = ppool.tile([128, 1024], F8, tag="p", name=f"pt{s}")
            na = 1024 - DVE_Q
            nc.scalar.activation(pt[:, 0:na], sp[:, 0:na], EXP, scale=0.125,
                                 bias=ln8[:])
            if DVE_Q:
                nc.vector.tensor_scalar(
                    pt[:, na:1024].bitcast(I8), sp[:, na:1024],
                    SCHRAUD_MUL, SCHRAUD_BIAS,
                    mybir.AluOpType.mult, mybir.AluOpType.add)
            for h in range(2):
                nc.tensor.matmul(
                    Os[h][:],
                    lhsT=v_pair(j, 2 * p + h),
                    rhs=pt[:, h * 512:(h + 1) * 512].unsqueeze(1)
                        .broadcast_to([128, 2, 512]),
                    start=(j == 0), stop=(j == JT - 1),
                    perf_mode=DR)
            if j == JT - 1:
                final = s == len(steps) - 1
                recs, osb_t = [], []
                for h in range(2):
                    rec = small.tile([1, 512], F, tag="rec", name=f"rec{s}_{h}")
                    nc.vector.reciprocal(rec[:], Os[h][64:65, :])
                    recs.append(rec)
                    ocp = small.tile([64, 512], F, tag=f"oc{h}", name=f"oc{s}_{h}")
                    # fold the 1/8 pt scale into the O evacuation
                    if final:
                        nc.scalar.mul(ocp[:], Os[h][0:64, :], 0.125)
                    else:
                        nc.vector.tensor_scalar_mul(ocp[:], Os[h][0:64, :], 0.125)
                    osb_t.append(ocp)
                pending_norm = make_norm(p, qc, osb_t, recs, s, final=final)
        pending_norm()

    nc.compile()
    return nc


def _get_program():
    global _PROGRAM
    if _PROGRAM is None:
        _PROGRAM = _build_program()
    return _PROGRAM


def _prep_core_inputs(inputs, Wq, bq, Wk, bk, Wv, bv, core):
    b, g = divmod(core, 2)
    hs = slice(g * FPC, (g + 1) * FPC)
    xt = np.ascontiguousarray(inputs[b].T).astype(BF16)
    wqt = np.ascontiguousarray(Wq[hs, :].T).astype(BF16)
    wkt = np.ascontiguousarray(Wk[hs, :].T).astype(BF16)
    wvt = np.zeros((D, VW), dtype=BF16)
    bv_aug = np.zeros((1, VW), dtype=BF16)
    for l in range(HPC):
        gh = g * HPC + l
        wvt[:, l * 65:l * 65 + 64] = Wv[gh * 64:(gh + 1) * 64, :].T.astype(BF16)
        bv_aug[0, l * 65:l * 65 + 64] = bv[gh * 64:(gh + 1) * 64].astype(BF16)
        bv_aug[0, l * 65 + 64] = 0.125   # denom column: folds pt scale 8
    bqk = np.stack([np.asarray(bq[hs], dtype=np.float32),
                    np.asarray(bk[hs], dtype=np.float32)], axis=1)
    return {
        "xt": xt,
        "wqt": wqt,
        "wkt": wkt,
        "wvt": wvt,
        "bqk": np.ascontiguousarray(bqk),
        "bv": bv_aug,
    }


def kernel(inputs, Wq, bq, Wk, bk, Wv, bv, _trace=False):
    from concourse.bass_utils import run_bass_kernel_spmd

    inputs = np.asarray(inputs, dtype=np.float32)
    Wq, Wk, Wv = (np.asarray(w, dtype=np.float32) for w in (Wq, Wk, Wv))
    bq, bk, bv = (np.asarray(b, dtype=np.float32) for b in (bq, bk, bv))
    in_maps = [
        _prep_core_inputs(inputs, Wq, bq, Wk, bk, Wv, bv, c) for c in range(N_CORES)
    ]
    nc = _get_program()
    res = run_bass_kernel_spmd(nc, in_maps, list(range(N_CORES)), trace=_trace)
    full = np.empty((B, S, D), dtype=np.float32)
    for c in range(N_CORES):
        b, g = divmod(c, 2)
        full[b, :, g * FPC:(g + 1) * FPC] = res.results[c]["out"].T
    if _trace:
        return full, res
    return full


# revision 12
# speedup vs baseline: 1.1355x; 1.1355x over previous
"""Multi-head attention (B=4, S=2048, D=768, 12 heads) on 8 TRN2 NeuronCores.

Sharding: data parallel over batch (4) x tensor parallel over heads (2 groups
of 6 heads) = 8 cores. Each core computes its (batch, head-group) slice.

v5 pipeline (fp8-e3m4 flash attention):
  Q^T/K^T projections in [feat, seq] bf16 layout; V projected per seq-tile
  and evacuated once to fp8 e3m4 (4 mantissa bits) with a 0.125 ones column
  per head for the softmax denominator. Transposed scores S^T[k,q] per head
  pair with row-tiled concurrent K=64 bf16 matmuls; exp(0.125 s)*0.75
  written straight to e3m4: the leading q-columns on the scalar engine (ACT
  exp), the tail on the vector engine via a Schraudolph uint8 bit-trick
  (z*16/ln2 + bias, round-to-nearest, saturate-at-0, bitcast e3m4). P@V
  runs as plain fp8 matmuls - 1-byte moving data streams ~2 cols/cycle, so
  fp8 matmuls run ~1.8x faster than bf16 without any perf mode -
  accumulating O^T + denominator row in PSUM. fp32 reciprocal; broadcast
  and normalization multiply on GPSIMD (Pool), with the 1/8 score scale
  folded into the O evacuation.
Host side only reshapes/casts for sharding and un-transposes on gather.
"""

import numpy as np
import ml_dtypes

B, S, D = 4, 2048, 768
NH, HD = 12, 64
HPC = 6                 # heads per core
FPC = HPC * HD          # 384 features per core
VW = HPC * 65           # 390: V width with per-head denom column (PSUM)
VSLOT = 72              # e3 V slot stride per head (even byte offsets)
N_CORES = 8
BF16 = ml_dtypes.bfloat16

# exp constants: pt = 0.75*exp(0.125*s) in fp8 e3m4
PT_C = 0.75
PT_LN_SCALE = float(np.log(PT_C))
SCHRAUD_MUL = 0.125 * 16.0 / float(np.log(2.0))
SCHRAUD_SIGMA = 0.4
SCHRAUD_BIAS = 16.0 * (float(np.log2(PT_C)) + 3.0) - SCHRAUD_SIGMA
DVE_Q = 256             # of each 1024-col pt tile, last DVE_Q columns exp on
                        # DVE (head 1 tail) -> dve share = DVE_Q/1024

_PROGRAM = None


def _build_program():
    import concourse.bass as bass  # noqa: F401
    import concourse.mybir as mybir
    from concourse import bacc
    from concourse.tile import TileContext
    from contextlib import ExitStack

    F = mybir.dt.float32
    BF = mybir.dt.bfloat16
    E3 = mybir.dt.float8e3
    U8 = mybir.dt.uint8
    EXP = mybir.ActivationFunctionType.Exp

    nc = bacc.Bacc("TRN2", target_bir_lowering=False, debug=False, num_devices=N_CORES)

    xt = nc.dram_tensor("xt", [D, S], BF, kind="ExternalInput")
    wqt = nc.dram_tensor("wqt", [D, FPC], BF, kind="ExternalInput")
    wkt = nc.dram_tensor("wkt", [D, FPC], BF, kind="ExternalInput")
    wvt = nc.dram_tensor("wvt", [D, VW], BF, kind="ExternalInput")
    bqk = nc.dram_tensor("bqk", [FPC, 2], F, kind="ExternalInput")
    bv = nc.dram_tensor("bv", [1, VW], BF, kind="ExternalInput")
    out = nc.dram_tensor("out", [FPC, S], F, kind="ExternalOutput")

    KT = D // 128        # 6 contraction tiles for projections
    MT = FPC // 128      # 3 feature tiles (= head pairs)
    QC = S // 512        # 4 seq chunks of 512
    JT = S // 128        # 16 key tiles

    with TileContext(nc) as tc, ExitStack() as ctx:
        const = ctx.enter_context(tc.tile_pool(name="const", bufs=1))
        qkv = ctx.enter_context(tc.tile_pool(name="qkv", bufs=1))
        osb = ctx.enter_context(tc.tile_pool(name="osb", bufs=1))
        ppool = ctx.enter_context(tc.tile_pool(name="pt", bufs=8))
        small = ctx.enter_context(tc.tile_pool(name="small", bufs=4))
        ps_pr = ctx.enter_context(tc.tile_pool(name="pspr", bufs=2, space="PSUM"))
        ps_s = ctx.enter_context(tc.tile_pool(name="pss", bufs=2, space="PSUM"))
        ps_o = ctx.enter_context(tc.tile_pool(name="pso", bufs=1, space="PSUM"))

        # ---- stage inputs in SBUF with one consolidated DMA per tensor
        xt_all = const.tile([128, KT * S], BF, tag="xta", name="xta")
        xt_s = [xt_all[:, i * S:(i + 1) * S] for i in range(KT)]
        wq_all = const.tile([128, KT * FPC], BF, tag="wqa", name="wqa")
        wqt_s = [wq_all[:, i * FPC:(i + 1) * FPC] for i in range(KT)]
        wk_all = const.tile([128, KT * FPC], BF, tag="wka", name="wka")
        wkt_s = [wk_all[:, i * FPC:(i + 1) * FPC] for i in range(KT)]
        wv_all = const.tile([128, KT * VW], BF, tag="wva", name="wva")
        wvt_s = [wv_all[:, i * VW:(i + 1) * VW] for i in range(KT)]
        bqk_all = const.tile([128, MT * 2], F, tag="bqk", name="bqka")
        bq_s = [bqk_all[:, 2 * t_i:2 * t_i + 1] for t_i in range(MT)]
        bk_s = [bqk_all[:, 2 * t_i + 1:2 * t_i + 2] for t_i in range(MT)]
        bv_s = const.tile([1, VW], BF, tag="bv")

        nc.sync.dma_start(
            wq_all[:].rearrange("p (b c) -> p b c", b=KT),
            wqt[:].rearrange("(b p) c -> p b c", p=128))
        nc.sync.dma_start(
            wk_all[:].rearrange("p (b c) -> p b c", b=KT),
            wkt[:].rearrange("(b p) c -> p b c", p=128))
        nc.sync.dma_start(
            bqk_all[:].rearrange("p (t c) -> p t c", t=MT),
            bqk[:].rearrange("(t p) c -> p t c", p=128))
        ones_s = const.tile([1, 128], BF, tag="ones")
        nc.vector.memset(ones_s[:], 1.0)
        lnc = const.tile([128, 1], F, tag="lnc")
        nc.vector.memset(lnc[:], PT_LN_SCALE)

        # dummy exp so the ACT table set loads during the DMA prologue
        dummy = small.tile([1, 1], F, tag="dummy", name="dummy")
        nc.scalar.activation(dummy[:], bq_s[0][0:1, :], EXP)

        # e3m4 V tiles: [128 seq, 6 head slots of 72], ones col at 64
        v_s = [qkv.tile([128, HPC * VSLOT], E3, tag=f"v{m}", name=f"v{m}")
               for m in range(JT)]
        qt_s = [qkv.tile([128, S], BF, tag=f"q{t_i}", name=f"qt{t_i}") for t_i in range(MT)]
        kt_s = [qkv.tile([128, S], BF, tag=f"k{t_i}", name=f"kt{t_i}") for t_i in range(MT)]
        o_s = [osb.tile([128, S], F, tag=f"o{t_i}", name=f"ot{t_i}") for t_i in range(MT)]

        # ---- projection work, broken into single-matmul thunks
        def v_group_thunks(m):
            cell = {}

            def mk(kk):
                def thunk():
                    if "ps" not in cell:
                        cell["ps"] = ps_pr.tile([128, 512], F, tag="pr",
                                                name=f"psv{m}")
                    if kk < KT:
                        nc.tensor.matmul(
                            cell["ps"][:, 0:VW],
                            lhsT=xt_s[kk][:, m * 128:(m + 1) * 128],
                            rhs=wvt_s[kk][:], start=(kk == 0), stop=False)
                    else:
                        nc.tensor.matmul(cell["ps"][:, 0:VW], lhsT=ones_s[:],
                                         rhs=bv_s[:], start=False, stop=True)
                        dst = v_s[m][:].rearrange(
                            "p (h c) -> p h c", h=HPC)[:, :, 0:65]
                        src = cell["ps"][:, 0:VW].rearrange(
                            "p (h c) -> p h c", h=HPC)
                        nc.vector.tensor_copy(dst, src)
                return thunk
            return [mk(kk) for kk in range(KT + 1)]

        def qk_group_thunks(w_s, b_s, dst, p, qc, use_act=False):
            cell = {}

            def mk(kk):
                def thunk():
                    if "ps" not in cell:
                        cell["ps"] = ps_pr.tile([128, 512], F, tag="pr",
                                                name=f"psp{p}_{qc}")
                    nc.tensor.matmul(
                        cell["ps"][:],
                        lhsT=w_s[kk][:, p * 128:(p + 1) * 128],
                        rhs=xt_s[kk][:, qc * 512:(qc + 1) * 512],
                        start=(kk == 0), stop=(kk == KT - 1))
                    if kk == KT - 1:
                        if use_act:
                            nc.scalar.add(
                                dst[p][:, qc * 512:(qc + 1) * 512],
                                cell["ps"][:], b_s[p][:])
                        else:
                            nc.vector.tensor_scalar_add(
                                dst[p][:, qc * 512:(qc + 1) * 512],
                                cell["ps"][:], b_s[p][:])
                return thunk
            return [mk(kk) for kk in range(KT)]

        # xt chunk DMAs via SWDGE in parallel with weight DMAs on HWDGE
        for qc in range(QC):
            nc.gpsimd.dma_start(
                xt_all[:].rearrange("p (b c) -> p b c", b=KT)[
                    :, :, qc * 512:(qc + 1) * 512],
                xt[:].rearrange("(b p) c -> p b c", p=128)[
                    :, :, qc * 512:(qc + 1) * 512])
        nc.sync.dma_start(
            wv_all[:].rearrange("p (b c) -> p b c", b=KT),
            wvt[:].rearrange("(b p) c -> p b c", p=128))
        nc.sync.dma_start(bv_s[:], bv[:])

        # prologue compute: pair-0 chunk-0 Q/K projections
        for w_s, b_s, dst in ((wqt_s, bq_s, qt_s), (wkt_s, bk_s, kt_s)):
            for th in qk_group_thunks(w_s, b_s, dst, 0, qc=0):
                th()

        from collections import deque
        items = []   # (deadline, order, thunk)

        def add(deadline, thunks):
            for th in thunks:
                items.append((deadline, len(items), th))

        for m in range(JT):
            add(m, v_group_thunks(m))                       # PV(0,0,m) at step m
        for qc in range(1, QC):
            add(max(0, 4 * qc - 2),
                qk_group_thunks(wkt_s, bk_s, kt_s, 0, qc))
            add(max(0, 16 * qc - 2),
                qk_group_thunks(wqt_s, bq_s, qt_s, 0, qc))
        for p in range(1, MT):
            base = 64 * p
            for qc in range(QC):
                add(base + 4 * qc - 2,
                    qk_group_thunks(wkt_s, bk_s, kt_s, p, qc))
                add(base + 16 * qc - 2,
                    qk_group_thunks(wqt_s, bq_s, qt_s, p, qc))
        items.sort(key=lambda x: (x[0], x[1]))
        proj_q = deque(items)

        # ---- attention pipeline over flattened (pair, q-chunk, k-tile) steps
        steps = [(p, qc, j) for p in range(MT) for qc in range(QC)
                 for j in range(JT)]

        def emit_scores(p, qc, j):
            sp = ps_s.tile([128, 1024], F, tag="s", name=f"s{p}_{qc}_{j}")
            for h in range(2):
                nc.tensor.matmul(
                    sp[:, h * 512:(h + 1) * 512],
                    lhsT=kt_s[p][h * 64:(h + 1) * 64, j * 128:(j + 1) * 128],
                    rhs=qt_s[p][h * 64:(h + 1) * 64, qc * 512:(qc + 1) * 512],
                    start=True, stop=True, tile_position=(h * 64, 0))
            return sp

        def make_norm(p, qc, osb_t, recs, s, final=False):
            # broadcast + normalize on GPSIMD (Pool), freeing PE and DVE
            def norm():
                for h in range(2):
                    bc_sb = small.tile([64, 512], F, tag="bc", name=f"bcs{s}_{h}")
                    nc.gpsimd.partition_broadcast(bc_sb[:], recs[h][:],
                                                  channels=64)
                    nc.gpsimd.tensor_mul(
                        o_s[p][h * 64:(h + 1) * 64, qc * 512:(qc + 1) * 512],
                        osb_t[h][:], bc_sb[:])
                nc.sync.dma_start(
                    out[p * 128:(p + 1) * 128, qc * 512:(qc + 1) * 512],
                    o_s[p][:, qc * 512:(qc + 1) * 512])
            return norm

        sp_next = emit_scores(*steps[0])
        Os = None
        pending_norm = None
        for s, (p, qc, j) in enumerate(steps):
            sp = sp_next
            if s + 1 < len(steps):
                sp_next = emit_scores(*steps[s + 1])
            if pending_norm is not None:
                pending_norm()
                pending_norm = None
            extra = 2
            while proj_q and (proj_q[0][0] <= s or extra > 0):
                if proj_q[0][0] > s:
                    extra -= 1
                proj_q.popleft()[2]()
            if j == 0:
                O0 = ps_o.tile([65, 512], F, tag="o0", name=f"o0_{p}_{qc}")
                O1 = ps_o.tile([65, 512], F, tag="o1", name=f"o1_{p}_{qc}")
                Os = (O0, O1)
            # exp -> e3m4 pt: ACT takes the first 1024-DVE_Q columns, DVE
            # (Schraudolph uint8 bit-trick) the rest
            pt = ppool.tile([128, 1024], E3, tag="p", name=f"pt{s}")
            na = 1024 - DVE_Q
            nc.scalar.activation(pt[:, 0:na], sp[:, 0:na], EXP, scale=0.125,
                                 bias=lnc[:])
            if DVE_Q:
                nc.vector.tensor_scalar(
                    pt[:, na:1024].bitcast(U8), sp[:, na:1024],
                    SCHRAUD_MUL, SCHRAUD_BIAS,
                    mybir.AluOpType.mult, mybir.AluOpType.add)
            for h in range(2):
                lh = 2 * p + h
                nc.tensor.matmul(
                    Os[h][:],
                    lhsT=v_s[j][:, lh * VSLOT:lh * VSLOT + 65],
                    rhs=pt[:, h * 512:(h + 1) * 512],
                    start=(j == 0), stop=(j == JT - 1))
            if j == JT - 1:
                final = s == len(steps) - 1
                recs, osb_t = [], []
                for h in range(2):
                    rec = small.tile([1, 512], F, tag="rec", name=f"rec{s}_{h}")
                    nc.vector.reciprocal(rec[:], Os[h][64:65, :])
                    recs.append(rec)
                    ocp = small.tile([64, 512], F, tag=f"oc{h}", name=f"oc{s}_{h}")
                    # fold the 1/8 score scale into the O evacuation
                    if final:
                        nc.scalar.mul(ocp[:], Os[h][0:64, :], 0.125)
                    else:
                        nc.vector.tensor_scalar_mul(ocp[:], Os[h][0:64, :], 0.125)
                    osb_t.append(ocp)
                pending_norm = make_norm(p, qc, osb_t, recs, s, final=final)
        pending_norm()

    nc.compile()
    return nc


def _get_program():
    global _PROGRAM
    if _PROGRAM is None:
        _PROGRAM = _build_program()
    return _PROGRAM


def _prep_core_inputs(inputs, Wq, bq, Wk, bk, Wv, bv, core):
    b, g = divmod(core, 2)
    hs = slice(g * FPC, (g + 1) * FPC)
    xt = np.ascontiguousarray(inputs[b].T).astype(BF16)
    wqt = np.ascontiguousarray(Wq[hs, :].T).astype(BF16)
    wkt = np.ascontiguousarray(Wk[hs, :].T).astype(BF16)
    wvt = np.zeros((D, VW), dtype=BF16)
    bv_aug = np.zeros((1, VW), dtype=BF16)
    for l in range(HPC):
        gh = g * HPC + l
        wvt[:, l * 65:l * 65 + 64] = Wv[gh * 64:(gh + 1) * 64, :].T.astype(BF16)
        bv_aug[0, l * 65:l * 65 + 64] = bv[gh * 64:(gh + 1) * 64].astype(BF16)
        bv_aug[0, l * 65 + 64] = 0.125   # denom column: folds the 1/8 scale
    bqk = np.stack([np.asarray(bq[hs], dtype=np.float32),
                    np.asarray(bk[hs], dtype=np.float32)], axis=1)
    return {
        "xt": xt,
        "wqt": wqt,
        "wkt": wkt,
        "wvt": wvt,
        "bqk": np.ascontiguousarray(bqk),
        "bv": bv_aug,
    }


def kernel(inputs, Wq, bq, Wk, bk, Wv, bv, _trace=False):
    from concourse.bass_utils import run_bass_kernel_spmd

    inputs = np.asarray(inputs, dtype=np.float32)
    Wq, Wk, Wv = (np.asarray(w, dtype=np.float32) for w in (Wq, Wk, Wv))
    bq, bk, bv = (np.asarray(b, dtype=np.float32) for b in (bq, bk, bv))
    in_maps = [
        _prep_core_inputs(inputs, Wq, bq, Wk, bk, Wv, bv, c) for c in range(N_CORES)
    ]
    nc = _get_program()
    res = run_bass_kernel_spmd(nc, in_maps, list(range(N_CORES)), trace=_trace)
    full = np.empty((B, S, D), dtype=np.float32)
    for c in range(N_CORES):
        b, g = divmod(c, 2)
        full[b, :, g * FPC:(g + 1) * FPC] = res.results[c]["out"].T
    if _trace:
        return full, res
    return full
